# revision 32
# baseline (speedup 1.0000x reference)
# Triplet-margin loss kernel for Trainium2 (Bass/Tile), batch-sharded
# across 8 NeuronCores.
#
# reference math (torch F.pairwise_distance semantics):
#   d_ap[b,p] = || anc[b] - pos[b,p] + eps ||_2
#   d_an[b,n] = || anc[b] - neg[b,n] + eps ||_2
#   loss = mean_{b,p,n} max(d_ap[b,p] - d_an[b,n] + margin, 0)
#
# Final design (all op costs measured on HW):
#   Per 128-row batch-tile there are 24 distance columns ("slices"), in
#   neg0..neg15,pos0..pos7 order, streamed as 8 single + 7 double + 2
#   single [128,1024] fp32 chunks on the sync HWDGE queue (~420 GB/s).
#   Per slice:
#     DVE stt:  ring = (x - eps) - anc   fp32->fp16, no accum, ~1143ns
#               (ring of 16 fp16 buffers; back-to-back writes to the
#               same buffer stall ~2x, so never reuse adjacently)
#     ACT:      Square(ring)+accum -> d2 col, ~1367ns incl the serial
#               ACTIVATION_READ_ACCUMULATOR (no 16-bit fast path on ACT)
#   A few sums run on DVE instead (stt bypass/mult + accum, ~1267ns,
#   into a separate d2b tile - sharing ACT's d2 tile serializes DVE
#   sums behind ACT accumulator reads): one mid-stream pos col per tile
#   plus tile-1's last two slices (the true tail).  This balances
#   DVE ~ ACT ~ 65us busy, which IS the roofline for this op set:
#   2 passes/slice minimum, reduce-class ops have no DVE 2x mode,
#   tensor_tensor_reduce crashes this HW, and GpSimd help is net
#   negative (its concurrency inflates DVE/ACT/DMA durations ~20% at
#   density; even 8 sparse ops measurably lose).
#   d_an = sqrt(d2[0:16]) [128,16] mid-tile; d_ap = sqrt over the pos
#   cols once per tile (contiguous runs from d2 and d2b).  Pairing on
#   DVE: lp[:,p] = sum_n min(d_an - d_ap[p], 1) with a ones tile (host
#   computes hinge = 16 - lp), so no margin tensor and no Relu table.
#   ACT's table (Square+Sqrt) is primed by tiny ops during the DMA ramp
#   (otherwise a 1.3us ACT_TABLE_LOAD lands mid-stream at first Sqrt).
#   anc0 + the first two chunks ride the scalar (ACT) HWDGE queue so
#   the cold DMA ramp fills both queues; anc1's trigger is emitted
#   mid-tile-0 in ACT's program (a sync-queue anc1 put a 1.2us bubble
#   at the tile boundary).  Tile-1 reuses tile-0's 17 chunk buffers
#   (emission order makes the WAR visible to the tile framework).
#   Tile-0's pairing is emitted a few slices into tile-1's diff stream.
#   Fixed framework overhead (preamble + all-engine teardown) measures
#   ~12.9us regardless of kernel content; exec ~= content + 12.9us.
#   Run-to-run variance (~85 vs ~96us) tracks DMA contention across the
#   8 SPMD cores, not kernel scheduling.

import numpy as np

import concourse.bacc as bacc
import concourse.mybir as mybir
import concourse.tile as tile
from concourse import bass_utils

B, Z = 2048, 1024
NUM_POS, NUM_NEG = 8, 16
NJ = NUM_POS + NUM_NEG
MARGIN, EPS = 1.0, 1e-6
N_CORES = 8
BL = B // N_CORES  # 256 rows of anc per core
P = 128
NT = BL // P  # 2 batch-tiles per core
RING = 16
N_SINGLES = 8  # lead single-slice chunks per tile
PAIR_T0_AT = 9  # tile-0 pairing emitted after this many tile-1 diffs

GP_DIFF = ()  # gpsimd unused: its concurrency inflates DVE/ACT/DMA ~20%
# sums on DVE: a few pos-range slices mid-stream (hidden by ACT's queue
# lag) plus tile-1's true tail; they write d2b, never ACT's d2 tile
# (sharing one d2 tile serializes DVE sums behind ACT accumulator reads)
DVE_SUM = {0: (16,), 1: (16, 22, 23)}

F32 = mybir.dt.float32
FP16 = mybir.dt.float16
AF = mybir.ActivationFunctionType
OP = mybir.AluOpType

# chunk list: (first_slice, n_slices) in the neg-first slice order;
# trailing singles shorten the end-of-stream serial tail
CHUNKS = (
    [(j, 1) for j in range(N_SINGLES)]
    + [(j, 2) for j in range(N_SINGLES, NJ - 2, 2)]
    + [(NJ - 2, 1), (NJ - 1, 1)]
)
NCH = len(CHUNKS)


def _emit(tc, nc, anc, pos, neg, out):
    v = nc.vector
    act = nc.scalar
    gp = nc.gpsimd
    pos2 = pos.rearrange("(b j) z -> b (j z)", j=NUM_POS)  # [BL, 8*Z]
    neg2 = neg.rearrange("(b j) z -> b (j z)", j=NUM_NEG)  # [BL, 16*Z]

    def chunk_src(t, jj0, nsl):
        b0 = t * P
        if jj0 < NUM_NEG:
            return neg2[b0 : b0 + P, jj0 * Z : (jj0 + nsl) * Z]
        return pos2[b0 : b0 + P, (jj0 - NUM_NEG) * Z : (jj0 - NUM_NEG + nsl) * Z]

    CHUNK_OF = {}  # slice -> (chunk index, offset-within-chunk)
    for c, (jj0, nsl) in enumerate(CHUNKS):
        for q in range(nsl):
            CHUNK_OF[jj0 + q] = (c, q)

    with (
        tc.tile_pool(name="xp", bufs=1) as xp,
        tc.tile_pool(name="rp", bufs=1) as rp,
        tc.tile_pool(name="sp", bufs=1) as sp,
    ):
        xt = [xp.tile([P, 2 * Z], F32, name=f"xt{c}") for c in range(NCH)]
        ring = [rp.tile([P, Z], FP16, name=f"ring{r}") for r in range(RING)]
        act_scr = sp.tile([P, Z], FP16, name="act_scr")
        sq_scr = [sp.tile([P, Z], FP16, name=f"sq_scr{i}") for i in range(2)]
        ts_scr = [sp.tile([P, NUM_NEG], F32, name=f"ts{i}") for i in range(2)]
        ones_n = sp.tile([P, NUM_NEG], F32, name="ones_n")
        prime = sp.tile([P, 2], F32, name="prime")
        ancs = [sp.tile([P, Z], F32, name=f"anc{t}") for t in range(NT)]
        d2 = [sp.tile([P, NJ], F32, name=f"d2_{t}") for t in range(NT)]
        d2b = [sp.tile([P, 4], F32, name=f"d2b{t}") for t in range(NT)]
        dan = [sp.tile([P, NUM_NEG], F32, name=f"dan{t}") for t in range(NT)]
        dap = [sp.tile([P, NUM_POS], F32, name=f"dap{t}") for t in range(NT)]
        lp = [sp.tile([P, NUM_POS], F32, name=f"lp{t}") for t in range(NT)]

        v.memset(ones_n[:, :], 1.0)

        # prime the ACT function table (Square+Sqrt) during the DMA ramp
        act.activation(prime[:, 0:1], ones_n[:, 0:1], AF.Square)
        act.activation(prime[:, 1:2], ones_n[:, 0:1], AF.Sqrt)

        # anc0 first on the sync queue, then tile-0's chunks.  (tile-1's
        # chunk DMAs are emitted AFTER tile-0's compute so the framework
        # sees the WAR on the shared xt buffers.)
        Q10_CHUNKS = (9, 12)  # 1MB pair-chunks carried by the scalar queue
        nc.scalar.dma_start(ancs[0][:, :], anc[0:P, :])
        for c, (jj0, nsl) in enumerate(CHUNKS):
            if c in Q10_CHUNKS:
                continue
            nc.sync.dma_start(xt[c][:, 0 : nsl * Z], chunk_src(0, jj0, nsl))


        def dve_diff(t, jj, r):
            c, q = CHUNK_OF[jj]
            v.scalar_tensor_tensor(
                out=ring[r][:, :],
                in0=xt[c][:, q * Z : (q + 1) * Z],
                scalar=EPS,
                in1=ancs[t][:, :],
                op0=OP.subtract,
                op1=OP.subtract,
            )

        def gp_diff(t, jj, r):
            c, q = CHUNK_OF[jj]
            gp.tensor_tensor(
                out=ring[r][:, :],
                in0=xt[c][:, q * Z : (q + 1) * Z],
                in1=aprime[t][:, :],
                op=OP.subtract,
            )

        def act_sum(t, jj, r):
            act.activation(
                act_scr[:, :], ring[r][:, :], AF.Square,
                accum_out=d2[t][:, jj : jj + 1],
            )

        sqi = [0]
        D2B_COL = {
            (t, jj): i for t in range(NT) for i, jj in enumerate(DVE_SUM[t])
        }

        def dve_sum(t, jj, r):
            col = D2B_COL[(t, jj)]
            v.scalar_tensor_tensor(
                out=sq_scr[sqi[0] % 2][:, :],
                in0=ring[r][:, :],
                scalar=1.0,
                in1=ring[r][:, :],
                op0=OP.bypass,
                op1=OP.mult,
                accum_out=d2b[t][:, col : col + 1],
            )
            sqi[0] += 1

        def sqrt_neg(t):
            act.activation(dan[t][:, :], d2[t][:, 0:NUM_NEG], AF.Sqrt)

        def sqrt_pos(t):
            # ACT-summed pos cols from d2; DVE-summed pos cols from d2b.
            # Pos cols of tile t: 16..23; DVE_SUM[t] cols live in d2b.
            dve_cols = [jj for jj in DVE_SUM[t] if jj >= NUM_NEG]
            act_cols = [jj for jj in range(NUM_NEG, NJ) if jj not in DVE_SUM[t]]
            # contiguous runs for act cols (they are contiguous by design)
            a0, a1 = act_cols[0] - NUM_NEG, act_cols[-1] - NUM_NEG + 1
            act.activation(
                dap[t][:, a0:a1], d2[t][:, act_cols[0] : act_cols[-1] + 1], AF.Sqrt
            )
            for jj in dve_cols:
                col = D2B_COL[(t, jj)]
                act.activation(
                    dap[t][:, jj - NUM_NEG : jj - NUM_NEG + 1],
                    d2b[t][:, col : col + 1],
                    AF.Sqrt,
                )

        def pairing(t, p_i):
            # lp[:,p] = sum_n min(d_an - d_ap[p], 1); hinge = 16 - lp on host
            v.scalar_tensor_tensor(
                out=ts_scr[p_i % 2][:, :],
                in0=dan[t][:, :],
                scalar=dap[t][:, p_i : p_i + 1],
                op0=OP.subtract,
                in1=ones_n[:, :],
                op1=OP.min,
                accum_out=lp[t][:, p_i : p_i + 1],
            )

        slot = {}
        nxt = [0]

        def assign_slot(t, jj):
            r = nxt[0] % RING
            nxt[0] += 1
            slot[(t, jj)] = r
            return r

        # Sums for gpsimd-diffed slices are deferred ~3 slices so neither
        # ACT nor DVE head-of-line-blocks on the 3us gpsimd diff latency.
        gidx = [0]
        dve_pending = []  # (emit_gidx, t, jj)
        act_pending = []

        def flush_ready(force=False):
            while dve_pending and (force or gidx[0] - dve_pending[0][0] >= 1):
                _, t_, jj_ = dve_pending.pop(0)
                dve_sum(t_, jj_, slot[(t_, jj_)])
                if not force:
                    break
            while act_pending and (force or gidx[0] - act_pending[0][0] >= 3):
                _, t_, jj_ = act_pending.pop(0)
                act_sum(t_, jj_, slot[(t_, jj_)])
                if not force:
                    break

        def do_slice(t, jj):
            r = assign_slot(t, jj)
            if jj in GP_DIFF:
                gp_diff(t, jj, r)
            else:
                dve_diff(t, jj, r)
            if jj in DVE_SUM[t]:
                dve_pending.append((gidx[0], t, jj))
            elif jj in GP_DIFF:
                act_pending.append((gidx[0], t, jj))
            else:
                act_sum(t, jj, r)
            gidx[0] += 1
            flush_ready()
            # all 16 neg sums must be emitted before sqrt_neg
            if jj == NUM_NEG:
                flush_neg(t)
                sqrt_neg(t)

        def flush_neg(t):
            # force-emit any pending sums for neg slices of tile t
            for lst, fn in ((dve_pending, dve_sum), (act_pending, act_sum)):
                keep = []
                for g, t_, jj_ in lst:
                    if t_ == t and jj_ < NUM_NEG:
                        fn(t_, jj_, slot[(t_, jj_)])
                    else:
                        keep.append((g, t_, jj_))
                lst[:] = keep

        # ---- tile 0 ----
        for jj in range(NJ):
            do_slice(0, jj)
            # tile-0's q10 chunks: triggered a couple of slices in (not at
            # t=0, which would steal cold-ramp bandwidth from the first
            # sync-queue chunks); buffers are free so ACT never stalls
            if jj == 2:
                c = 9
                nc.scalar.dma_start(xt[c][:, 0 : CHUNKS[c][1] * Z], chunk_src(0, *CHUNKS[c]))
            if jj == 4:
                c = 12
                nc.scalar.dma_start(xt[c][:, 0 : CHUNKS[c][1] * Z], chunk_src(0, *CHUNKS[c]))
            if jj == 10:
                # anc1 rides the scalar queue concurrently with the chunk
                # stream (one ~0.7us ACT trigger mid-tile-0 instead of a
                # 1.2us bubble at the tile boundary on the sync queue)
                nc.scalar.dma_start(ancs[1][:, :], anc[P : 2 * P, :])
        flush_ready(force=True)
        sqrt_pos(0)

        # tile-1 DMAs (after tile-0 compute emission: WAR on shared xt)
        for c, (jj0, nsl) in enumerate(CHUNKS):
            if c in Q10_CHUNKS:
                continue
            nc.sync.dma_start(xt[c][:, 0 : nsl * Z], chunk_src(1, jj0, nsl))

        # ---- tile 1 diffs, with tile-0 pairing inserted mid-stream ----
        for jj in range(PAIR_T0_AT):
            do_slice(1, jj)
        for p_i in range(NUM_POS):
            pairing(0, p_i)
        for jj in range(PAIR_T0_AT, NJ):
            # t1's q10 chunks: triggers must be emitted BEFORE the diffs of
            # their own slices; by the time ACT reaches these positions its
            # queue is ~8us past tile-0's consumption of the same buffers
            if jj == 10:
                c = 9
                nc.scalar.dma_start(xt[c][:, 0 : CHUNKS[c][1] * Z], chunk_src(1, *CHUNKS[c]))
            if jj == 14:
                c = 12
                nc.scalar.dma_start(xt[c][:, 0 : CHUNKS[c][1] * Z], chunk_src(1, *CHUNKS[c]))
            do_slice(1, jj)
        flush_ready(force=True)
        sqrt_pos(1)
        for p_i in range(NUM_POS):
            pairing(1, p_i)
        nc.sync.dma_start(out[:, 0:NUM_POS], lp[0][:, :])
        nc.sync.dma_start(out[:, NUM_POS : 2 * NUM_POS], lp[1][:, :])


_NC_CACHE = None


def build():
    global _NC_CACHE
    if _NC_CACHE is None:
        nc = bacc.Bacc(
            "TRN2", target_bir_lowering=False, debug=False, num_devices=N_CORES
        )
        anc = nc.dram_tensor("anc", (BL, Z), F32, kind="ExternalInput").ap()
        pos = nc.dram_tensor("pos", (BL * NUM_POS, Z), F32, kind="ExternalInput").ap()
        neg = nc.dram_tensor("neg", (BL * NUM_NEG, Z), F32, kind="ExternalInput").ap()
        out = nc.dram_tensor("out", (P, NT * NUM_POS), F32, kind="ExternalOutput").ap()
        with tile.TileContext(nc) as tc:
            _emit(tc, nc, anc, pos, neg, out)
        nc.compile()
        _NC_CACHE = nc
    return _NC_CACHE


def make_in_maps(anc_embedding, pos_embedding, neg_embedding):
    anc_embedding = np.asarray(anc_embedding, dtype=np.float32)
    pos_embedding = np.asarray(pos_embedding, dtype=np.float32)
    neg_embedding = np.asarray(neg_embedding, dtype=np.float32)
    in_maps = []
    for c in range(N_CORES):
        in_maps.append(
            {
                "anc": np.ascontiguousarray(anc_embedding[c * BL : (c + 1) * BL]),
                "pos": np.ascontiguousarray(
                    pos_embedding[c * BL * NUM_POS : (c + 1) * BL * NUM_POS]
                ),
                "neg": np.ascontiguousarray(
                    neg_embedding[c * BL * NUM_NEG : (c + 1) * BL * NUM_NEG]
                ),
            }
        )
    return in_maps


def combine(outs):
    # each lp entry holds sum_n min(d_an - d_ap, 1) for one (row, pos) pair;
    # hinge sum for that pair = NUM_NEG - lp.  loss = mean over all pairs/negs.
    total_pairs = B * NUM_POS
    s = 0.0
    for o in outs:
        s += o.astype(np.float64).sum()
    return np.float32((NUM_NEG * total_pairs - s) / (B * NUM_POS * NUM_NEG))


def kernel(anc_embedding, pos_embedding, neg_embedding):
    nc = build()
    in_maps = make_in_maps(anc_embedding, pos_embedding, neg_embedding)
    res = bass_utils.run_bass_kernel_spmd(nc, in_maps, core_ids=list(range(N_CORES)))
    return combine([r["out"] for r in res.results])


# revision 33
# speedup vs baseline: 1.1316x; 1.1316x over previous
# Triplet-margin loss kernel for Trainium2 (Bass/Tile), batch-sharded
# across 8 NeuronCores.
#
# reference math (torch F.pairwise_distance semantics):
#   d_ap[b,p] = || anc[b] - pos[b,p] + eps ||_2
#   d_an[b,n] = || anc[b] - neg[b,n] + eps ||_2
#   loss = mean_{b,p,n} max(d_ap[b,p] - d_an[b,n] + margin, 0)
#
# Final design (all op costs measured on HW):
#   Per 128-row batch-tile there are 24 distance columns ("slices"), in
#   neg0..neg15,pos0..pos7 order, streamed as 8 single + 7 double + 2
#   single [128,1024] fp32 chunks on the sync HWDGE queue (~420 GB/s).
#   Per slice:
#     DVE stt:  ring = (x - eps) - anc   fp32->fp16, no accum, ~1143ns
#               (ring of 16 fp16 buffers; back-to-back writes to the
#               same buffer stall ~2x, so never reuse adjacently)
#     ACT:      Square(ring)+accum -> d2 col, ~1367ns incl the serial
#               ACTIVATION_READ_ACCUMULATOR (no 16-bit fast path on ACT)
#   A few sums run on DVE instead (stt bypass/mult + accum, ~1267ns,
#   into a separate d2b tile - sharing ACT's d2 tile serializes DVE
#   sums behind ACT accumulator reads): one mid-stream pos col per tile
#   plus tile-1's last two slices (the true tail).  This balances
#   DVE ~ ACT ~ 65us busy, which IS the roofline for this op set:
#   2 passes/slice minimum, reduce-class ops have no DVE 2x mode,
#   tensor_tensor_reduce crashes this HW, and GpSimd help is net
#   negative (its concurrency inflates DVE/ACT/DMA durations ~20% at
#   density; even 8 sparse ops measurably lose).
#   d_an = sqrt(d2[0:16]) [128,16] mid-tile; d_ap = sqrt over the pos
#   cols once per tile (contiguous runs from d2 and d2b).  Pairing on
#   DVE: lp[:,p] = sum_n min(d_an - d_ap[p], 1) with a ones tile (host
#   computes hinge = 16 - lp), so no margin tensor and no Relu table.
#   ACT's table (Square+Sqrt) is primed by tiny ops during the DMA ramp
#   (otherwise a 1.3us ACT_TABLE_LOAD lands mid-stream at first Sqrt).
#   anc0 + the first two chunks ride the scalar (ACT) HWDGE queue so
#   the cold DMA ramp fills both queues; anc1's trigger is emitted
#   mid-tile-0 in ACT's program (a sync-queue anc1 put a 1.2us bubble
#   at the tile boundary).  Tile-1 reuses tile-0's 17 chunk buffers
#   (emission order makes the WAR visible to the tile framework).
#   Tile-0's pairing is emitted a few slices into tile-1's diff stream.
#   Fixed framework overhead (preamble + all-engine teardown) measures
#   ~12.9us regardless of kernel content; exec ~= content + 12.9us.
#   Run-to-run variance (~85 vs ~96us) tracks DMA contention across the
#   8 SPMD cores, not kernel scheduling.

import numpy as np

import concourse.bacc as bacc
import concourse.mybir as mybir
import concourse.tile as tile
from concourse import bass_utils

B, Z = 2048, 1024
NUM_POS, NUM_NEG = 8, 16
NJ = NUM_POS + NUM_NEG
MARGIN, EPS = 1.0, 1e-6
N_CORES = 8
BL = B // N_CORES  # 256 rows of anc per core
P = 128
NT = BL // P  # 2 batch-tiles per core
RING = 16
N_SINGLES = 8  # lead single-slice chunks per tile
PAIR_T0_AT = 9  # tile-0 pairing emitted after this many tile-1 diffs

GP_DIFF = ()  # gpsimd unused: its concurrency inflates DVE/ACT/DMA ~20%
# sums on DVE: a few pos-range slices mid-stream (hidden by ACT's queue
# lag) plus tile-1's true tail; they write d2b, never ACT's d2 tile
# (sharing one d2 tile serializes DVE sums behind ACT accumulator reads)
DVE_SUM = {0: (16,), 1: (16, 22, 23)}

F32 = mybir.dt.float32
FP16 = mybir.dt.float16
AF = mybir.ActivationFunctionType
OP = mybir.AluOpType

# chunk list: (first_slice, n_slices) in the neg-first slice order;
# trailing singles shorten the end-of-stream serial tail
CHUNKS = (
    [(j, 1) for j in range(N_SINGLES)]
    + [(j, 2) for j in range(N_SINGLES, NJ - 2, 2)]
    + [(NJ - 2, 1), (NJ - 1, 1)]
)
NCH = len(CHUNKS)


def _emit(tc, nc, anc, pos, neg, out):
    v = nc.vector
    act = nc.scalar
    gp = nc.gpsimd
    pos2 = pos.rearrange("(b j) z -> b (j z)", j=NUM_POS)  # [BL, 8*Z]
    neg2 = neg.rearrange("(b j) z -> b (j z)", j=NUM_NEG)  # [BL, 16*Z]

    def chunk_src(t, jj0, nsl):
        b0 = t * P
        if jj0 < NUM_NEG:
            return neg2[b0 : b0 + P, jj0 * Z : (jj0 + nsl) * Z]
        return pos2[b0 : b0 + P, (jj0 - NUM_NEG) * Z : (jj0 - NUM_NEG + nsl) * Z]

    CHUNK_OF = {}  # slice -> (chunk index, offset-within-chunk)
    for c, (jj0, nsl) in enumerate(CHUNKS):
        for q in range(nsl):
            CHUNK_OF[jj0 + q] = (c, q)

    with (
        tc.tile_pool(name="xp", bufs=1) as xp,
        tc.tile_pool(name="rp", bufs=1) as rp,
        tc.tile_pool(name="sp", bufs=1) as sp,
    ):
        xt = [xp.tile([P, 2 * Z], F32, name=f"xt{c}") for c in range(NCH)]
        ring = [rp.tile([P, Z], FP16, name=f"ring{r}") for r in range(RING)]
        act_scr = sp.tile([P, Z], FP16, name="act_scr")
        sq_scr = [sp.tile([P, Z], FP16, name=f"sq_scr{i}") for i in range(2)]
        ts_scr = [sp.tile([P, NUM_NEG], F32, name=f"ts{i}") for i in range(2)]
        ones_n = sp.tile([P, NUM_NEG], F32, name="ones_n")
        prime = sp.tile([P, 2], F32, name="prime")
        ancs = [sp.tile([P, Z], F32, name=f"anc{t}") for t in range(NT)]
        d2 = [sp.tile([P, NJ], F32, name=f"d2_{t}") for t in range(NT)]
        d2b = [sp.tile([P, 4], F32, name=f"d2b{t}") for t in range(NT)]
        dan = [sp.tile([P, NUM_NEG], F32, name=f"dan{t}") for t in range(NT)]
        dap = [sp.tile([P, NUM_POS], F32, name=f"dap{t}") for t in range(NT)]
        lp = [sp.tile([P, NUM_POS], F32, name=f"lp{t}") for t in range(NT)]

        v.memset(ones_n[:, :], 1.0)

        # prime the ACT function table (Square+Sqrt) during the DMA ramp
        act.activation(prime[:, 0:1], ones_n[:, 0:1], AF.Square)
        act.activation(prime[:, 1:2], ones_n[:, 0:1], AF.Sqrt)

        # anc0 first on the sync queue, then tile-0's chunks.  (tile-1's
        # chunk DMAs are emitted AFTER tile-0's compute so the framework
        # sees the WAR on the shared xt buffers.)
        nc.scalar.dma_start(ancs[0][:, :], anc[0:P, :])
        for c, (jj0, nsl) in enumerate(CHUNKS):
            nc.sync.dma_start(xt[c][:, 0 : nsl * Z], chunk_src(0, jj0, nsl))


        def dve_diff(t, jj, r):
            c, q = CHUNK_OF[jj]
            v.scalar_tensor_tensor(
                out=ring[r][:, :],
                in0=xt[c][:, q * Z : (q + 1) * Z],
                scalar=EPS,
                in1=ancs[t][:, :],
                op0=OP.subtract,
                op1=OP.subtract,
            )

        def gp_diff(t, jj, r):
            c, q = CHUNK_OF[jj]
            gp.tensor_tensor(
                out=ring[r][:, :],
                in0=xt[c][:, q * Z : (q + 1) * Z],
                in1=aprime[t][:, :],
                op=OP.subtract,
            )

        def act_sum(t, jj, r):
            act.activation(
                act_scr[:, :], ring[r][:, :], AF.Square,
                accum_out=d2[t][:, jj : jj + 1],
            )

        sqi = [0]
        D2B_COL = {
            (t, jj): i for t in range(NT) for i, jj in enumerate(DVE_SUM[t])
        }

        def dve_sum(t, jj, r):
            col = D2B_COL[(t, jj)]
            v.scalar_tensor_tensor(
                out=sq_scr[sqi[0] % 2][:, :],
                in0=ring[r][:, :],
                scalar=1.0,
                in1=ring[r][:, :],
                op0=OP.bypass,
                op1=OP.mult,
                accum_out=d2b[t][:, col : col + 1],
            )
            sqi[0] += 1

        def sqrt_neg(t):
            act.activation(dan[t][:, :], d2[t][:, 0:NUM_NEG], AF.Sqrt)

        def sqrt_pos(t):
            # ACT-summed pos cols from d2; DVE-summed pos cols from d2b.
            # Pos cols of tile t: 16..23; DVE_SUM[t] cols live in d2b.
            dve_cols = [jj for jj in DVE_SUM[t] if jj >= NUM_NEG]
            act_cols = [jj for jj in range(NUM_NEG, NJ) if jj not in DVE_SUM[t]]
            # contiguous runs for act cols (they are contiguous by design)
            a0, a1 = act_cols[0] - NUM_NEG, act_cols[-1] - NUM_NEG + 1
            act.activation(
                dap[t][:, a0:a1], d2[t][:, act_cols[0] : act_cols[-1] + 1], AF.Sqrt
            )
            for jj in dve_cols:
                col = D2B_COL[(t, jj)]
                act.activation(
                    dap[t][:, jj - NUM_NEG : jj - NUM_NEG + 1],
                    d2b[t][:, col : col + 1],
                    AF.Sqrt,
                )

        def pairing(t, p_i):
            # lp[:,p] = sum_n min(d_an - d_ap[p], 1); hinge = 16 - lp on host
            v.scalar_tensor_tensor(
                out=ts_scr[p_i % 2][:, :],
                in0=dan[t][:, :],
                scalar=dap[t][:, p_i : p_i + 1],
                op0=OP.subtract,
                in1=ones_n[:, :],
                op1=OP.min,
                accum_out=lp[t][:, p_i : p_i + 1],
            )

        slot = {}
        nxt = [0]

        def assign_slot(t, jj):
            r = nxt[0] % RING
            nxt[0] += 1
            slot[(t, jj)] = r
            return r

        # Sums for gpsimd-diffed slices are deferred ~3 slices so neither
        # ACT nor DVE head-of-line-blocks on the 3us gpsimd diff latency.
        gidx = [0]
        dve_pending = []  # (emit_gidx, t, jj)
        act_pending = []

        def flush_ready(force=False):
            while dve_pending and (force or gidx[0] - dve_pending[0][0] >= 1):
                _, t_, jj_ = dve_pending.pop(0)
                dve_sum(t_, jj_, slot[(t_, jj_)])
                if not force:
                    break
            while act_pending and (force or gidx[0] - act_pending[0][0] >= 3):
                _, t_, jj_ = act_pending.pop(0)
                act_sum(t_, jj_, slot[(t_, jj_)])
                if not force:
                    break

        def do_slice(t, jj):
            r = assign_slot(t, jj)
            if jj in GP_DIFF:
                gp_diff(t, jj, r)
            else:
                dve_diff(t, jj, r)
            if jj in DVE_SUM[t]:
                dve_pending.append((gidx[0], t, jj))
            elif jj in GP_DIFF:
                act_pending.append((gidx[0], t, jj))
            else:
                act_sum(t, jj, r)
            gidx[0] += 1
            flush_ready()
            # all 16 neg sums must be emitted before sqrt_neg
            if jj == NUM_NEG:
                flush_neg(t)
                sqrt_neg(t)

        def flush_neg(t):
            # force-emit any pending sums for neg slices of tile t
            for lst, fn in ((dve_pending, dve_sum), (act_pending, act_sum)):
                keep = []
                for g, t_, jj_ in lst:
                    if t_ == t and jj_ < NUM_NEG:
                        fn(t_, jj_, slot[(t_, jj_)])
                    else:
                        keep.append((g, t_, jj_))
                lst[:] = keep

        # ---- tile 0 ----
        for jj in range(NJ):
            do_slice(0, jj)
            if jj == 10:
                # anc1 rides the scalar queue concurrently with the chunk
                # stream (one ~0.7us ACT trigger mid-tile-0 instead of a
                # 1.2us bubble at the tile boundary on the sync queue)
                nc.scalar.dma_start(ancs[1][:, :], anc[P : 2 * P, :])
        flush_ready(force=True)
        sqrt_pos(0)

        # tile-1 DMAs (after tile-0 compute emission: WAR on shared xt)
        for c, (jj0, nsl) in enumerate(CHUNKS):
            nc.sync.dma_start(xt[c][:, 0 : nsl * Z], chunk_src(1, jj0, nsl))

        # ---- tile 1 diffs, with tile-0 pairing inserted mid-stream ----
        for jj in range(PAIR_T0_AT):
            do_slice(1, jj)
        for p_i in range(NUM_POS):
            pairing(0, p_i)
        for jj in range(PAIR_T0_AT, NJ):
            do_slice(1, jj)
        flush_ready(force=True)
        sqrt_pos(1)
        for p_i in range(NUM_POS):
            pairing(1, p_i)
        nc.sync.dma_start(out[:, 0:NUM_POS], lp[0][:, :])
        nc.sync.dma_start(out[:, NUM_POS : 2 * NUM_POS], lp[1][:, :])


_NC_CACHE = None


def build():
    global _NC_CACHE
    if _NC_CACHE is None:
        nc = bacc.Bacc(
            "TRN2", target_bir_lowering=False, debug=False, num_devices=N_CORES
        )
        anc = nc.dram_tensor("anc", (BL, Z), F32, kind="ExternalInput").ap()
        pos = nc.dram_tensor("pos", (BL * NUM_POS, Z), F32, kind="ExternalInput").ap()
        neg = nc.dram_tensor("neg", (BL * NUM_NEG, Z), F32, kind="ExternalInput").ap()
        out = nc.dram_tensor("out", (P, NT * NUM_POS), F32, kind="ExternalOutput").ap()
        with tile.TileContext(nc) as tc:
            _emit(tc, nc, anc, pos, neg, out)
        nc.compile()
        _NC_CACHE = nc
    return _NC_CACHE


def make_in_maps(anc_embedding, pos_embedding, neg_embedding):
    anc_embedding = np.asarray(anc_embedding, dtype=np.float32)
    pos_embedding = np.asarray(pos_embedding, dtype=np.float32)
    neg_embedding = np.asarray(neg_embedding, dtype=np.float32)
    in_maps = []
    for c in range(N_CORES):
        in_maps.append(
            {
                "anc": np.ascontiguousarray(anc_embedding[c * BL : (c + 1) * BL]),
                "pos": np.ascontiguousarray(
                    pos_embedding[c * BL * NUM_POS : (c + 1) * BL * NUM_POS]
                ),
                "neg": np.ascontiguousarray(
                    neg_embedding[c * BL * NUM_NEG : (c + 1) * BL * NUM_NEG]
                ),
            }
        )
    return in_maps


def combine(outs):
    # each lp entry holds sum_n min(d_an - d_ap, 1) for one (row, pos) pair;
    # hinge sum for that pair = NUM_NEG - lp.  loss = mean over all pairs/negs.
    total_pairs = B * NUM_POS
    s = 0.0
    for o in outs:
        s += o.astype(np.float64).sum()
    return np.float32((NUM_NEG * total_pairs - s) / (B * NUM_POS * NUM_NEG))


def kernel(anc_embedding, pos_embedding, neg_embedding):
    nc = build()
    in_maps = make_in_maps(anc_embedding, pos_embedding, neg_embedding)
    res = bass_utils.run_bass_kernel_spmd(nc, in_maps, core_ids=list(range(N_CORES)))
    return combine([r["out"] for r in res.results])
